# revision 20
# baseline (speedup 1.0000x reference)
"""Trainium2 Bass kernel for nn_Binarization (2-order masked residual binarization).

Computes, per row (output channel) of x (masked by `mask`):
    m = mask; cnt = sum(m); mean1 = sum(x*m)/max(cnt,1)
    c1 = (x - mean1)*m; s1 = sum(|c1|)/cnt; g = sign(c1)
    b1 = s1*g + mean1;  residual2 = (x*m - b1*m)*m
    mean2 = sum(residual2)/cnt; c2 = (residual2 - mean2)*m
    s2 = sum(|c2|)/cnt; h = sign(c2)
    out = (b1 + s2*h + mean2) * m

Implementation notes:
 - Full inputs are sharded by rows across 8 NeuronCores (row stats are
   per-row, so shards are independent).
 - Per core, rows are processed in 128-row tiles with the full 4096-column
   row resident, so every row reduction is a single fused accumulate.
 - The pipeline runs on masked data xm = x*m; out-of-mask lanes carry
   known per-row constants, so full-row ACT reductions are corrected with
   closed-form per-row scalar terms instead of extra masked passes.
 - Engine split: DVE does the fused product+reduce / scalar_tensor_tensor
   passes, ACT does abs/sign passes (free accumulation), GPSIMD does the
   cheap tensor_scalar passes and the final mask multiply.
"""

import atexit
import numpy as np

import concourse.bacc as bacc
import concourse.mybir as mybir
from concourse import tile
from concourse.bass_utils import run_bass_kernel_spmd

OP = mybir.AluOpType
AF = mybir.ActivationFunctionType
F32 = mybir.dt.float32
BF16 = mybir.dt.bfloat16
U8 = mybir.dt.uint8

N_CORES = 8
ROWS = 11008
COLS = 4096
ROWS_PER_CORE = ROWS // N_CORES  # 1376

# engine assignment knobs (tuned on hardware)
CNT_ON = "act"    # 'act' | 'dve'   (gpsimd tensor_scalar rejected by walrus)
Q_ON = "dve"      # 'dve' | 'act'
OUT_ON = "gp"     # 'gp' | 'dve'
F_INPLACE = True  # write F over q's buffer
JUNK_MODE = "bitcast"  # 'bitcast' | 'dedicated'


def build_nc(rows=ROWS_PER_CORE, cols=COLS, repeat=1):
    """repeat>1 wraps the whole tile sweep in a runtime loop (same data, same
    output) — used only to measure steady-state HW kernel time via wall-clock
    deltas, since NTFF profiling is unavailable in this environment."""
    import contextlib

    nc = bacc.Bacc(None, target_bir_lowering=False, debug=False)
    x = nc.declare_dram_parameter("x", [rows, cols], F32, isOutput=False)
    mask = nc.declare_dram_parameter("mask", [rows, cols], U8, isOutput=False)
    out = nc.declare_dram_parameter("out", [rows, cols], F32, isOutput=True)
    ntiles = (rows + 127) // 128
    fcols = float(cols)

    with tile.TileContext(nc) as tc:
        with tc.tile_pool(name="xp", bufs=2) as xp, \
             tc.tile_pool(name="mp", bufs=2) as mp, \
             tc.tile_pool(name="xo", bufs=3) as xo, \
             tc.tile_pool(name="gup", bufs=2) as gup, \
             tc.tile_pool(name="c2p", bufs=2) as c2p, \
             tc.tile_pool(name="hup", bufs=2) as hup, \
             tc.tile_pool(name="qp", bufs=2) as qp, \
             tc.tile_pool(name="cz", bufs=1) as cz, \
             tc.tile_pool(name="stp", bufs=72) as stp:

            zero1 = cz.tile([128, 1], F32, tag="zero1")
            nc.vector.memset(zero1[:], 0.0)

            rep_ctx = tc.For_i(0, repeat, 1) if repeat > 1 else None
            if rep_ctx is not None:
                rep_ctx.__enter__()
            for i in range(ntiles):
                r0 = i * 128
                p = min(128, rows - r0)

                x_t = xp.tile([p, cols], F32, tag="x")
                m_t = mp.tile([p, cols], U8, tag="m")
                nc.sync.dma_start(out=x_t[:], in_=x[r0:r0 + p, :])
                nc.sync.dma_start(out=m_t[:], in_=mask[r0:r0 + p, :])

                st_n = [0]

                def st():
                    st_n[0] += 1
                    return stp.tile([p, 1], F32, tag="st", name=f"st{i}_{st_n[0]}")

                # q/F tile also hosts the junk byte-regions for ACT reductions
                q_t = qp.tile([p, cols], F32, tag="q", name=f"q_{i}")
                if JUNK_MODE == "bitcast":
                    junkA = q_t.bitcast(U8)[:, :cols]
                    junkB = q_t.bitcast(U8)[:, cols:2 * cols]
                    junkC = q_t.bitcast(U8)[:, 2 * cols:3 * cols]
                else:
                    jt = qp.tile([p, cols], U8, tag="junk", name=f"junk_{i}")
                    junkA = jt[:, :]
                    junkB = jt[:, :]
                    junkC = jt[:, :]

                # p1: xm = (x*1)*m with fused row-sum S1
                # (tensor_tensor_reduce is unsupported on this runtime; stt's
                #  accum_out provides the same fused product+rowsum)
                xm_t = xo.tile([p, cols], F32, tag="xo")
                S1 = st()
                nc.vector.scalar_tensor_tensor(
                    out=xm_t[:], in0=x_t[:], scalar=1.0, in1=m_t[:],
                    op0=OP.mult, op1=OP.mult, accum_out=S1[:])

                # p2: cnt = rowsum(m)  (junk elementwise out)
                cnt = st()
                if CNT_ON == "act":
                    nc.scalar.activation(out=junkC, in_=m_t[:], func=AF.Copy,
                                         accum_out=cnt[:])
                else:
                    cj = stp.tile([p, 1], F32, tag="cj")
                    nc.vector.tensor_scalar(cj.broadcast_to([p, cols]), m_t[:], 1.0,
                                            0.0, OP.mult, OP.add, accum_out=cnt[:])

                # tiny stats chain (DVE unless noted)
                safe_cnt = st()
                nc.vector.tensor_scalar(safe_cnt[:], cnt[:], 1.0, None, OP.max)
                inv = st()
                nc.vector.reciprocal(inv[:], safe_cnt[:])
                mean1 = st()
                nc.vector.tensor_tensor(out=mean1[:], in0=S1[:], in1=inv[:], op=OP.mult)
                negmean1 = st()
                nc.vector.tensor_scalar(negmean1[:], mean1[:], -1.0, None, OP.mult)
                cntmC = st()  # cnt - C  (<= 0)
                nc.vector.tensor_scalar(cntmC[:], cnt[:], fcols, None, OP.subtract)
                absm1 = st()  # |mean1| = max(-mean1, mean1)
                nc.vector.tensor_scalar(absm1[:], mean1[:], -1.0, mean1[:],
                                        OP.mult, OP.max)

                # p3 [ACT]: A1' = sum|xm - mean1| (full row); junk out
                A1p = st()
                nc.scalar.activation(out=junkA, in_=xm_t[:], func=AF.Abs,
                                     bias=negmean1[:], accum_out=A1p[:])

                # p4 [ACT]: gu = sign(xm - mean1), P' = rowsum(gu)
                gu_t = gup.tile([p, cols], BF16, tag="gu")
                Pp = st()
                nc.scalar.activation(out=gu_t[:], in_=xm_t[:], func=AF.Sign,
                                     bias=negmean1[:], accum_out=Pp[:])
                # sgu = sign(0 - mean1): out-of-mask value of gu
                sgu = st()
                nc.scalar.activation(out=sgu[:], in_=zero1[:p, :], func=AF.Sign,
                                     bias=negmean1[:])

                # A1 = A1' + (cnt-C)*|mean1| ; s1 = A1/cnt
                A1 = st()
                nc.vector.scalar_tensor_tensor(out=A1[:], in0=cntmC[:], scalar=absm1[:],
                                               in1=A1p[:], op0=OP.mult, op1=OP.add)
                s1 = st()
                nc.vector.tensor_tensor(out=s1[:], in0=A1[:], in1=inv[:], op=OP.mult)
                negs1 = st()
                nc.vector.tensor_scalar(negs1[:], s1[:], -1.0, None, OP.mult)
                # P_masked = P' + (cnt-C)*sgu
                Pm = st()
                nc.vector.scalar_tensor_tensor(out=Pm[:], in0=cntmC[:], scalar=sgu[:],
                                               in1=Pp[:], op0=OP.mult, op1=OP.add)
                # mean2 = ((S1 - cnt*mean1) - s1*Pm)/cnt
                u1 = st()
                nc.vector.scalar_tensor_tensor(out=u1[:], in0=cnt[:], scalar=negmean1[:],
                                               in1=S1[:], op0=OP.mult, op1=OP.add)
                u2 = st()
                nc.vector.scalar_tensor_tensor(out=u2[:], in0=Pm[:], scalar=negs1[:],
                                               in1=u1[:], op0=OP.mult, op1=OP.add)
                mean2 = st()
                nc.vector.tensor_tensor(out=mean2[:], in0=u2[:], in1=inv[:], op=OP.mult)
                # K = mean1 + mean2, negK = -K
                negK = st()
                nc.vector.tensor_scalar(negK[:], mean1[:], mean2[:], -1.0,
                                        OP.add, OP.mult)
                K = st()
                nc.vector.tensor_scalar(K[:], negK[:], -1.0, None, OP.mult)

                # p5 [DVE]: c2X = xm - s1*gu   (== c2 + K on masked lanes)
                c2_t = c2p.tile([p, cols], F32, tag="c2")
                nc.vector.scalar_tensor_tensor(out=c2_t[:], in0=gu_t[:], scalar=negs1[:],
                                               in1=xm_t[:], op0=OP.mult, op1=OP.add)

                # out-of-mask value of c2X and its A2 contribution
                oc2 = st()
                nc.vector.tensor_tensor(out=oc2[:], in0=negs1[:], in1=sgu[:], op=OP.mult)
                voc = st()  # oc2 - K
                nc.vector.tensor_tensor(out=voc[:], in0=oc2[:], in1=negK[:], op=OP.add)
                a2c = st()  # |oc2 - K| = max(-(oc2-K), oc2-K)
                nc.vector.tensor_scalar(a2c[:], voc[:], -1.0, voc[:], OP.mult, OP.max)

                # p6 [ACT]: A2' = sum|c2X - K| ; junk out
                A2p = st()
                nc.scalar.activation(out=junkB, in_=c2_t[:], func=AF.Abs,
                                     bias=negK[:], accum_out=A2p[:])
                A2 = st()
                nc.vector.scalar_tensor_tensor(out=A2[:], in0=cntmC[:], scalar=a2c[:],
                                               in1=A2p[:], op0=OP.mult, op1=OP.add)
                s2 = st()
                nc.vector.tensor_tensor(out=s2[:], in0=A2[:], in1=inv[:], op=OP.mult)

                # p7 [ACT]: hu = sign(c2X - K)
                hu_t = hup.tile([p, cols], BF16, tag="hu")
                nc.scalar.activation(out=hu_t[:], in_=c2_t[:], func=AF.Sign,
                                     bias=negK[:])

                # p8: q = s2*hu + K
                if Q_ON == "act":
                    nc.scalar.activation(out=q_t[:], in_=hu_t[:], func=AF.Identity,
                                         bias=K[:], scale=s2[:])
                else:
                    nc.vector.tensor_scalar(q_t[:], hu_t[:], s2[:], K[:],
                                            OP.mult, OP.add)

                # p9 [DVE]: F = s1*gu + q
                if F_INPLACE:
                    f_t = q_t
                else:
                    f_t = qp.tile([p, cols], F32, tag="f")
                nc.vector.scalar_tensor_tensor(out=f_t[:], in0=gu_t[:], scalar=s1[:],
                                               in1=q_t[:], op0=OP.mult, op1=OP.add)

                # p10: out = F * m
                o_t = xo.tile([p, cols], F32, tag="xo")
                eng = nc.gpsimd if OUT_ON == "gp" else nc.vector
                eng.tensor_tensor(out=o_t[:], in0=f_t[:], in1=m_t[:], op=OP.mult)

                nc.sync.dma_start(out=out[r0:r0 + p, :], in_=o_t[:])

            if rep_ctx is not None:
                rep_ctx.__exit__(None, None, None)

    nc.finalize()
    return nc


_NC_CACHE = {}


def _get_nc(rows, cols):
    key = (rows, cols)
    if key not in _NC_CACHE:
        _NC_CACHE[key] = build_nc(rows, cols)
    return _NC_CACHE[key]


def kernel(x: np.ndarray, mask: np.ndarray) -> np.ndarray:
    assert x.shape == (ROWS, COLS) and mask.shape == (ROWS, COLS)
    x = np.ascontiguousarray(x, dtype=np.float32)
    m_u8 = np.ascontiguousarray(mask).view(np.uint8)

    nc = _get_nc(ROWS_PER_CORE, COLS)
    in_maps = []
    for c in range(N_CORES):
        sl = slice(c * ROWS_PER_CORE, (c + 1) * ROWS_PER_CORE)
        in_maps.append({"x": x[sl], "mask": m_u8[sl]})

    res = run_bass_kernel_spmd(nc, in_maps, list(range(N_CORES)))
    out = np.concatenate([res.results[c]["out"] for c in range(N_CORES)], axis=0)
    return out.astype(np.float32, copy=False)


if __name__ == "__main__":
    rng = np.random.default_rng(0)
    x = (rng.standard_normal((ROWS, COLS)) * 0.02).astype(np.float32)
    mask = rng.integers(0, 2, size=(ROWS, COLS)).astype(bool)
    out = kernel(x=x, mask=mask)
    print("out", out.shape, out.dtype, float(np.abs(out).max()))
